# revision 43
# baseline (speedup 1.0000x reference)
"""Dilated self-attention TRN2 Bass kernel (v4).

Problem (hardcoded): B=2, N=8192, C=256, WS=[2048,4096,8192], RS=[1,2,4],
HEAD_IDX=0 -> G=7 groups of s=2048 rows each.

Sharding: 8 cores, core d = (b=d//4, q=d%4) owns output positions
[2048q, 2048(q+1)) of batch b.  Jobs per core:
  A: the r=1 segment group of its quarter (2048 queries, causal)
  B: the 1024-query half of the r=2 group landing in its quarter
  C: the 512-query quarter of the r=4 group landing in its quarter
B/C use a uniform fake-prefix layout; per-core differences are data-only
(zero-padded slabs + exp bias that kills masked prefix k-tiles).

Design:
  - bf16 transport + bf16 projections; B/C queries and their diagonal k/v
    tiles are strided views of the A-quarter slabs (projected once).
  - Mixed-precision scores: k-tiles whose rows are guaranteed many-key on
    every core that uses them run as fp8e4 DoubleRow matmuls (4x fewer PE
    cycles); diagonal tiles covering few-key rows stay bf16.  Probs are bf16
    (unnormalized exp reaches e^25, beyond fp8/fp16 range).
  - PSUM-resident combine: per output 128-row tile, the C placement, B
    placement (f32r stages) and A prob@V chain accumulate into one PSUM
    tile; out = psum[:, 0:256] * 1/psum[:, 256] straight from PSUM.
  - Pair-merged exp on Act; fp8 slab copies on Pool/Act; bf16 on DVE.
"""

import os

import numpy as np

B, N, C = 2, 8192, 256
S = 2048
NCORES = 8
SCALE = 0.0625    # 1/sqrt(256)
NEG = -1.0e9

_PROG = None


def _build_program():
    import concourse.mybir as mybir
    import concourse.tile as tile
    from concourse import bacc

    F32 = mybir.dt.float32
    MMF = mybir.dt.float32r
    BF = mybir.dt.bfloat16
    F8 = mybir.dt.float8e4
    Exp = mybir.ActivationFunctionType.Exp
    DR = mybir.MatmulPerfMode.DoubleRow
    no_f8 = bool(os.environ.get("KERNEL_NO_F8"))

    nc = bacc.Bacc("TRN2", target_bir_lowering=False, debug=False,
                   num_devices=NCORES)

    xA_d = nc.dram_tensor("xA", [C, S], BF, kind="ExternalInput")
    xB_d = nc.dram_tensor("xB", [C, 1024], BF, kind="ExternalInput")
    xC_d = nc.dram_tensor("xC", [C, 1536], BF, kind="ExternalInput")
    w_d = nc.dram_tensor("w", [C, 3 * C], BF, kind="ExternalInput")
    bias_d = nc.dram_tensor("bias", [128, 16], F32, kind="ExternalInput")
    out_d = nc.dram_tensor("out", [S, C], F32, kind="ExternalOutput")

    with tile.TileContext(nc) as tc:
        with (
            tc.tile_pool(name="const", bufs=1) as cpool,
            tc.tile_pool(name="xsb", bufs=1) as xpool,
            tc.tile_pool(name="ksl", bufs=1) as kpool,
            tc.tile_pool(name="vext", bufs=1) as vpool,
            tc.tile_pool(name="probs", bufs=42) as ppool,
            tc.tile_pool(name="stage", bufs=1) as spool,
            tc.tile_pool(name="fin", bufs=8) as fpool,
            tc.tile_pool(name="rec", bufs=4) as rpool,
            tc.tile_pool(name="ps_s", bufs=2, space="PSUM") as ps_sc,
            tc.tile_pool(name="ps_u", bufs=2, space="PSUM") as ps_u,
            tc.tile_pool(name="ps_p", bufs=2, space="PSUM") as ps_proj,
        ):
            # ---- weights + xA (the A slab feeds most early PE work) ----
            wt = []
            for ci in range(2):
                t = cpool.tile([128, 3 * C], BF, tag=f"wt{ci}", name=f"wt{ci}")
                eng = nc.sync if ci == 0 else nc.scalar
                eng.dma_start(t[:, 0:512], w_d[128 * ci:128 * (ci + 1), 0:512])
                wt.append(t)
            w_sb = {}
            for i, nm in enumerate(("q", "k", "v")):
                for ci in range(2):
                    w_sb[nm, ci] = wt[ci][:, 256 * i:256 * (i + 1)]

            xA = []
            for ci in range(2):
                t = xpool.tile([128, S], BF, tag=f"xA{ci}", name=f"xA{ci}")
                eng = nc.sync if ci == 0 else nc.scalar
                eng.dma_start(t[:, 0:512], xA_d[128 * ci:128 * (ci + 1), 0:512])
                xA.append(t)
            bias_t = cpool.tile([128, 16], F32, tag="bias")
            nc.sync.dma_start(bias_t[:], bias_d[:])
            warm = cpool.tile([128, 1], F32, tag="warm")
            nc.scalar.activation(warm[:], bias_t[:, 0:1], Exp, bias=0.0,
                                 scale=0.0)
            for ci in range(2):
                eng = nc.sync if ci == 0 else nc.scalar
                eng.dma_start(wt[ci][:, 512:768],
                              w_d[128 * ci:128 * (ci + 1), 512:768])
            for ci in range(2):
                eng = nc.sync if ci == 0 else nc.scalar
                eng.dma_start(xA[ci][:, 512:S],
                              xA_d[128 * ci:128 * (ci + 1), 512:S])
            xC = []
            for ci in range(2):
                t = xpool.tile([128, 1536], BF, tag=f"xC{ci}", name=f"xC{ci}")
                eng = nc.sync if ci == 0 else nc.scalar
                eng.dma_start(t[:], xC_d[128 * ci:128 * (ci + 1), :])
                xC.append(t)
            xB = []
            for ci in range(2):
                t = xpool.tile([128, 1024], BF, tag=f"xB{ci}", name=f"xB{ci}")
                eng = nc.sync if ci == 0 else nc.scalar
                eng.dma_start(t[:], xB_d[128 * ci:128 * (ci + 1), :])
                xB.append(t)

            # ---- projection targets ----
            qt_bf = [kpool.tile([128, S], BF, tag=f"qtbf{c}", name=f"qtbf{c}")
                     for c in range(2)]
            qt_f8 = kpool.tile([128, 2, S], F8, tag="qtf8", name="qtf8")
            ktA_bf = [kpool.tile([128, S], BF, tag=f"kAbf{c}", name=f"kAbf{c}")
                      for c in range(2)]
            ktA_f8 = kpool.tile([128, 2, S], F8, tag="kAf8", name="kAf8")
            ktA_lo = kpool.tile([128, 2, S], F8, tag="kAlo", name="kAlo")
            ktB_f8 = kpool.tile([128, 2, 1024], F8, tag="kBf8", name="kBf8")
            ktC_f8 = kpool.tile([128, 2, 1536], F8, tag="kCf8", name="kCf8")
            ktB_bf = ktC_bf = None
            if no_f8:
                ktB_bf = [kpool.tile([128, 1024], BF, tag=f"kBbf{c}",
                                     name=f"kBbf{c}") for c in range(2)]
                ktC_bf = [kpool.tile([128, 1536], BF, tag=f"kCbf{c}",
                                     name=f"kCbf{c}") for c in range(2)]

            # persistent v slabs: [128, 16, 257] bf16 per job, col 256 = 1.0
            vsl = {}
            for jn in ("C", "B", "A"):
                v = vpool.tile([128, 16, 257], BF, tag=f"v{jn}", name=f"v{jn}")
                vsl[jn] = v
                nc.gpsimd.memset(v[:, 0:16, 256:257], 1.0)

            def vext(jn, t):
                return vsl[jn][:, t, :]

            # ---- projections (bf16) ----
            # psum -> bf16 slab (DVE) -> fp8 slab (Pool, SBUF->SBUF), or
            # psum -> fp8 slab directly on Act for fp8-only slabs.
            def proj_cols(dst_bf, dst_f8, wkey, src, c0, c1, dst_lo=None):
                for co in range(2):
                    ps = ps_proj.tile([128, 512], F32, tag="proj")
                    for ci in range(2):
                        nc.tensor.matmul(
                            ps[:, 0:c1 - c0],
                            w_sb[wkey, ci][:, 128 * co:128 * (co + 1)],
                            src[ci][:, c0:c1], start=(ci == 0), stop=(ci == 1))
                    if dst_bf is not None:
                        nc.vector.tensor_copy(dst_bf[co][:, c0:c1],
                                              ps[:, 0:c1 - c0])
                        if dst_f8 is not None:
                            if co == 0:
                                nc.gpsimd.tensor_copy(dst_f8[:, co, c0:c1],
                                                      dst_bf[co][:, c0:c1])
                            else:
                                nc.scalar.copy(dst_f8[:, co, c0:c1],
                                               dst_bf[co][:, c0:c1])
                            if dst_lo is not None:
                                eng = nc.vector if co == 0 else nc.gpsimd
                                eng.tensor_sub(dst_lo[:, co, c0:c1],
                                               dst_bf[co][:, c0:c1],
                                               dst_f8[:, co, c0:c1])
                    elif dst_f8 is not None:
                        nc.vector.tensor_copy(dst_f8[:, co, c0:c1],
                                              ps[:, 0:c1 - c0])

            def proj_v(jn, m, stat_aps2):
                """Project v tiles 2m, 2m+1 of job jn; one paired copy.
                C/B copies go on Act (idle during prep); A's on DVE."""
                ps = ps_proj.tile([128, 2, 256], F32, tag="proj", name="psv")
                for h in range(2):
                    for ci in range(2):
                        nc.tensor.matmul(ps[:, h, :], stat_aps2[h][ci],
                                         w_sb["v", ci][:],
                                         start=(ci == 0), stop=(ci == 1))
                nc.vector.tensor_copy(vsl[jn][:, 2 * m:2 * m + 2, 0:256],
                                      ps[:])

            # queries + A keys first (only need xA); then C, B keys; then V

            proj_cols(qt_bf, qt_f8, "q", xA, 0, 512)
            proj_cols(ktA_bf, ktA_f8, "k", xA, 0, 512, dst_lo=ktA_lo)
            for qc in range(1, 4):
                proj_cols(qt_bf, qt_f8, "q", xA, 512 * qc, 512 * (qc + 1))
                proj_cols(ktA_bf, ktA_f8, "k", xA, 512 * qc, 512 * (qc + 1),
                          dst_lo=ktA_lo)
            for kc in range(3):
                proj_cols(ktC_bf, ktC_f8, "k", xC, 512 * kc, 512 * (kc + 1))
            for kc in range(2):
                proj_cols(ktB_bf, ktB_f8, "k", xB, 512 * kc, 512 * (kc + 1))

            def xa_strided(t, base, step):
                off = (t - base) * 128 * step
                return [xA[ci][:, off:off + 128 * step:step] for ci in range(2)]

            def v_stat(jn, t):
                if jn == "C":
                    if t < 12:
                        return [xC[ci][:, 128 * t:128 * (t + 1)]
                                for ci in range(2)]
                    return xa_strided(t, 12, 4)
                if jn == "B":
                    if t < 8:
                        return [xB[ci][:, 128 * t:128 * (t + 1)]
                                for ci in range(2)]
                    return xa_strided(t, 8, 2)
                return [xA[ci][:, 128 * t:128 * (t + 1)] for ci in range(2)]


            for jn in ("C", "B"):
                for m in range(8):
                    proj_v(jn, m, [v_stat(jn, 2 * m), v_stat(jn, 2 * m + 1)])

            # ---- constants ----
            ones_t = cpool.tile([128, 128], F32, tag="ones")
            nc.gpsimd.memset(ones_t[:], 1.0)
            mtri_f = cpool.tile([128, 128], F32, tag="mtri_f")
            nc.gpsimd.affine_select(
                out=mtri_f[:], in_=ones_t[:],
                compare_op=mybir.AluOpType.is_ge,
                fill=0.0, base=0,
                pattern=[[1, 128]], channel_multiplier=-1,
            )
            mtri = cpool.tile([128, 128], BF, tag="mtri")
            nc.vector.tensor_copy(mtri[:], mtri_f[:])

            pmats = {}
            for stride, u in [(2, 0), (2, 1), (4, 0), (4, 1), (4, 2), (4, 3)]:
                pf = cpool.tile([128, 128], F32, tag=f"pmf{stride}_{u}",
                                name=f"pmf{stride}_{u}")
                nc.gpsimd.affine_select(
                    out=pf[:], in_=ones_t[:],
                    compare_op=mybir.AluOpType.is_equal,
                    fill=0.0, base=128 * u,
                    pattern=[[1, 128]], channel_multiplier=-stride,
                )
                pm = cpool.tile([128, 128], MMF, tag=f"pm{stride}_{u}",
                                name=f"pm{stride}_{u}")
                nc.vector.tensor_copy(pm[:], pf[:])
                pmats[stride, u] = pm

            # ---- attention ----
            def kt_f8_ap(jn, kt):
                if jn == "A":
                    return ktA_f8[:, :, 128 * kt:128 * (kt + 1)]
                if jn == "B":
                    if kt < 8:
                        return ktB_f8[:, :, 128 * kt:128 * (kt + 1)]
                    o = 256 * (kt - 8)
                    return ktA_f8[:, :, o:o + 256:2]
                if kt < 12:
                    return ktC_f8[:, :, 128 * kt:128 * (kt + 1)]
                o = 512 * (kt - 12)
                return ktA_f8[:, :, o:o + 512:4]

            def kt_bf_ap(jn, kt, ci):
                if jn == "A":
                    return ktA_bf[ci][:, 128 * kt:128 * (kt + 1)]
                if jn == "B":
                    if kt < 8:
                        return ktB_bf[ci][:, 128 * kt:128 * (kt + 1)]
                    o = 256 * (kt - 8)
                    return ktA_bf[ci][:, o:o + 256:2]
                if kt < 12:
                    return ktC_bf[ci][:, 128 * kt:128 * (kt + 1)]
                o = 512 * (kt - 12)
                return ktA_bf[ci][:, o:o + 512:4]

            def q_bf_ap(jn, i, qoff, ci):
                if jn == "A":
                    c0 = 512 * i + qoff
                    return qt_bf[ci][:, c0:512 * (i + 1)]
                if jn == "B":
                    c0 = 1024 * i + 2 * qoff
                    return qt_bf[ci][:, c0:1024 * (i + 1):2]
                return qt_bf[ci][:, 4 * qoff:S:4]

            def q_f8_ap(jn, i, qoff):
                if jn == "A":
                    c0 = 512 * i + qoff
                    return qt_f8[:, :, c0:512 * (i + 1)]
                if jn == "B":
                    c0 = 1024 * i + 2 * qoff
                    return qt_f8[:, :, c0:1024 * (i + 1):2]
                return qt_f8[:, :, 4 * qoff:S:4]

            def is_f8(jn, i, kt):
                if no_f8:
                    return False
                if jn == "A":
                    return i >= 1
                if jn == "B":
                    return i == 1 or kt < 8
                return kt < 12

            def bias_ap(jn, pair):
                if jn == "B" and pair < 4:
                    return bias_t[:, pair:pair + 1]
                if jn == "C" and pair < 6:
                    return bias_t[:, 4 + pair:5 + pair]
                return 0.0

            jobs = {"A": dict(P=0), "B": dict(P=8), "C": dict(P=12)}
            stB = [spool.tile([128, 258], MMF, tag=f"stB{t}", name=f"stB{t}")
                   for t in range(8)]
            stC = [spool.tile([128, 258], MMF, tag=f"stC{t}", name=f"stC{t}")
                   for t in range(4)]
            # stage col 257 is a dummy for the even-free-size f32r rule; it
            # only ever flows to psum col 257, which is never read.
            out_r = out_d.rearrange("(t p) c -> t p c", p=128)

            def scores_block(jn, i):
                """Emit scores+exp+mask for block i; return probs pair list."""
                P = jobs[jn]["P"]
                nkt = P + 4 * i + 4
                pbs = []
                for m in range(nkt // 2):
                    sc = ps_sc.tile([128, 2, 512], F32, tag="sc")
                    diag = False
                    qoffs = []
                    for h in range(2):
                        kt = 2 * m + h
                        jd = kt - (P + 4 * i)
                        qoff = max(0, 128 * jd)
                        qoffs.append(qoff)
                        diag = diag or jd >= 0
                        if is_f8(jn, i, kt):
                            klo = (ktA_lo[:, :, 128 * kt:128 * (kt + 1)]
                                   if jn == "A" else None)
                            nc.tensor.matmul(
                                sc[:, h, qoff:512], kt_f8_ap(jn, kt),
                                q_f8_ap(jn, i, qoff),
                                start=True, stop=(klo is None), perf_mode=DR)
                            if klo is not None:
                                nc.tensor.matmul(
                                    sc[:, h, qoff:512], klo,
                                    q_f8_ap(jn, i, qoff),
                                    start=False, stop=True, perf_mode=DR)
                        else:
                            for ci in range(2):
                                nc.tensor.matmul(
                                    sc[:, h, qoff:512], kt_bf_ap(jn, kt, ci),
                                    q_bf_ap(jn, i, qoff, ci),
                                    start=(ci == 0), stop=(ci == 1))
                    pb = ppool.tile([128, 2, 512], BF, tag="pb")
                    if sum(qoffs) > 256:
                        for h in range(2):
                            qoff = qoffs[h]
                            nc.scalar.activation(
                                pb[:, h, qoff:512], sc[:, h, qoff:512], Exp,
                                bias=0.0, scale=SCALE)
                    else:
                        nc.scalar.activation(pb[:], sc[:], Exp,
                                             bias=bias_ap(jn, m), scale=SCALE)
                    if diag:
                        for h in range(2):
                            jd = 2 * m + h - (P + 4 * i)
                            if jd >= 0:
                                qoff = qoffs[h]
                                nc.gpsimd.tensor_mul(
                                    pb[:, h, qoff:qoff + 128],
                                    pb[:, h, qoff:qoff + 128], mtri[:])
                    pbs.append(pb)
                return pbs

            def u_chain(jn, pbs, i, j, pre=()):
                P = jobs[jn]["P"]
                nk = P + 4 * i + j + 1
                u = ps_u.tile([128, 258], F32, tag="u")
                for n, (pm, st) in enumerate(pre):
                    nc.tensor.matmul(u[:], pm[:], st[:],
                                     start=(n == 0), stop=False)
                for kt in range(nk):
                    nc.tensor.matmul(
                        u[:, 0:257],
                        pbs[kt // 2][:, kt % 2, 128 * j:128 * (j + 1)],
                        vext(jn, kt),
                        start=(not pre and kt == 0), stop=(kt == nk - 1))
                return u

            # Emission order tuned so Act's exp queue (C, B0, B1) stays
            # ahead of the PE chains that consume the probs, while A's
            # scores fill PE gaps.
            for m in range(2):
                proj_v("A", m, [v_stat("A", 2 * m), v_stat("A", 2 * m + 1)])
            pbsC = scores_block("C", 0)
            pbA = {0: scores_block("A", 0)}
            pbsB = {0: scores_block("B", 0), 1: scores_block("B", 1)}
            for j in range(4):
                u = u_chain("C", pbsC, 0, j)
                nc.vector.tensor_copy(stC[j][:, 0:257], u[:, 0:257])
            for j in range(4):
                u = u_chain("B", pbsB[0], 0, j)
                nc.vector.tensor_copy(stB[j][:, 0:257], u[:, 0:257])
            for j in range(4):
                u = u_chain("B", pbsB[1], 1, j)
                nc.vector.tensor_copy(stB[4 + j][:, 0:257], u[:, 0:257])
            pbA[1] = scores_block("A", 1)

            # A job ascending; next block's scores + V tiles emitted before
            # this block's chains.  pb pool slot reuse: A3's tiles wrap onto
            # A0's slots (freed by chains t0-3, emitted earlier) -- safe with
            # bufs=40.
            for i in range(4):
                for m in (2 * i + 2, 2 * i + 3):
                    if m < 8:
                        proj_v("A", m, [v_stat("A", 2 * m),
                                        v_stat("A", 2 * m + 1)])
                if i < 2:
                    pbA[i + 2] = scores_block("A", i + 2)
                for j in (range(4) if i < 3 else (3, 2, 1, 0)):
                    t = 4 * i + j
                    u = u_chain("A", pbA[i], i, j,
                                pre=((pmats[4, t % 4], stC[t // 4]),
                                     (pmats[2, t % 2], stB[t // 2])))
                    rec = rpool.tile([128, 1], F32, tag="rec")
                    nc.vector.reciprocal(rec[:], u[:, 256:257])
                    fin = fpool.tile([128, 256], F32, tag="fin")
                    nc.vector.tensor_scalar_mul(fin[:], u[:, 0:256], rec[:])
                    eng = nc.sync if t % 2 == 0 else nc.scalar
                    eng.dma_start(out_r[t], fin[:])

    nc.compile()
    return nc


def _get_program():
    global _PROG
    if _PROG is None:
        _PROG = _build_program()
    return _PROG


def make_in_maps(x, Wq, Wk, Wv):
    """Host-side sharding: gather / transpose / zero-pad + bf16 transport."""
    import concourse.mybir as mybir
    BFNP = mybir.dt.np(mybir.dt.bfloat16)

    x = np.asarray(x, dtype=np.float32)
    w = np.concatenate([np.asarray(Wq, dtype=np.float32),
                        np.asarray(Wk, dtype=np.float32),
                        np.asarray(Wv, dtype=np.float32)], axis=1)
    w = np.ascontiguousarray(w).astype(BFNP)
    in_maps = []
    for d in range(NCORES):
        b, q = divmod(d, 4)
        xA = np.ascontiguousarray(x[b, 2048 * q:2048 * (q + 1), :].T)

        seg = 4096 * (q // 2)
        if q % 2 == 1:
            xB = np.ascontiguousarray(x[b, seg:seg + 2048:2, :].T)
        else:
            xB = np.zeros((C, 1024), np.float32)

        r0c = 512 * q
        grp4 = x[b, 0:8192:4, :]
        rowsC = np.concatenate(
            [grp4[0:r0c], np.zeros((1536 - r0c, C), np.float32)], axis=0)
        xC = np.ascontiguousarray(rowsC.T)

        bias = np.zeros((128, 16), np.float32)
        if q % 2 == 0:
            bias[:, 0:4] = NEG          # B prefix pairs 0..3 masked
        for p in range(6):
            if p >= 2 * q:
                bias[:, 4 + p] = NEG    # C prefix pairs >= 2q masked

        in_maps.append({"xA": xA.astype(BFNP), "xB": xB.astype(BFNP),
                        "xC": xC.astype(BFNP), "w": w, "bias": bias})
    return in_maps


def kernel(x, Wq, Wk, Wv):
    from concourse.bass_utils import run_bass_kernel_spmd

    nc = _get_program()
    in_maps = make_in_maps(x, Wq, Wk, Wv)
    res = run_bass_kernel_spmd(nc, in_maps, core_ids=list(range(NCORES)))
    out = np.empty((B, N, C), np.float32)
    for d in range(NCORES):
        b, q = divmod(d, 4)
        out[b, 2048 * q:2048 * (q + 1), :] = res.results[d]["out"]
    return out


# revision 47
# speedup vs baseline: 1.0058x; 1.0058x over previous
"""Dilated self-attention TRN2 Bass kernel (v4).

Problem (hardcoded): B=2, N=8192, C=256, WS=[2048,4096,8192], RS=[1,2,4],
HEAD_IDX=0 -> G=7 groups of s=2048 rows each.

Sharding: 8 cores, core d = (b=d//4, q=d%4) owns output positions
[2048q, 2048(q+1)) of batch b.  Jobs per core:
  A: the r=1 segment group of its quarter (2048 queries, causal)
  B: the 1024-query half of the r=2 group landing in its quarter
  C: the 512-query quarter of the r=4 group landing in its quarter
B/C use a uniform fake-prefix layout; per-core differences are data-only
(zero-padded slabs + exp bias that kills masked prefix k-tiles).

Design:
  - bf16 transport + bf16 projections; B/C queries and their diagonal k/v
    tiles are strided views of the A-quarter slabs (projected once).
  - Mixed-precision scores: k-tiles whose rows are guaranteed many-key on
    every core that uses them run as fp8e4 DoubleRow matmuls (4x fewer PE
    cycles); diagonal tiles covering few-key rows stay bf16.  Probs are bf16
    (unnormalized exp reaches e^25, beyond fp8/fp16 range).
  - PSUM-resident combine: per output 128-row tile, the C placement, B
    placement (f32r stages) and A prob@V chain accumulate into one PSUM
    tile; out = psum[:, 0:256] * 1/psum[:, 256] straight from PSUM.
  - Pair-merged exp on Act; fp8 slab copies on Pool/Act; bf16 on DVE.
"""

import os

import numpy as np

B, N, C = 2, 8192, 256
S = 2048
NCORES = 8
SCALE = 0.0625    # 1/sqrt(256)
NEG = -1.0e9

_PROG = None


def _build_program():
    import concourse.mybir as mybir
    import concourse.tile as tile
    from concourse import bacc

    F32 = mybir.dt.float32
    MMF = mybir.dt.float32r
    BF = mybir.dt.bfloat16
    F8 = mybir.dt.float8e4
    Exp = mybir.ActivationFunctionType.Exp
    DR = mybir.MatmulPerfMode.DoubleRow
    no_f8 = bool(os.environ.get("KERNEL_NO_F8"))

    nc = bacc.Bacc("TRN2", target_bir_lowering=False, debug=False,
                   num_devices=NCORES)

    xA_d = nc.dram_tensor("xA", [C, S], BF, kind="ExternalInput")
    xB_d = nc.dram_tensor("xB", [C, 1024], BF, kind="ExternalInput")
    xC_d = nc.dram_tensor("xC", [C, 1536], BF, kind="ExternalInput")
    w_d = nc.dram_tensor("w", [C, 3 * C], BF, kind="ExternalInput")
    bias_d = nc.dram_tensor("bias", [128, 16], F32, kind="ExternalInput")
    out_d = nc.dram_tensor("out", [S, C], F32, kind="ExternalOutput")

    with tile.TileContext(nc) as tc:
        with (
            tc.tile_pool(name="const", bufs=1) as cpool,
            tc.tile_pool(name="xsb", bufs=1) as xpool,
            tc.tile_pool(name="ksl", bufs=1) as kpool,
            tc.tile_pool(name="vext", bufs=1) as vpool,
            tc.tile_pool(name="probs", bufs=42) as ppool,
            tc.tile_pool(name="stage", bufs=1) as spool,
            tc.tile_pool(name="fin", bufs=8) as fpool,
            tc.tile_pool(name="rec", bufs=4) as rpool,
            tc.tile_pool(name="ps_s", bufs=2, space="PSUM") as ps_sc,
            tc.tile_pool(name="ps_u", bufs=2, space="PSUM") as ps_u,
            tc.tile_pool(name="ps_p", bufs=2, space="PSUM") as ps_proj,
        ):
            # ---- weights + xA (the A slab feeds most early PE work) ----
            wt = []
            for ci in range(2):
                t = cpool.tile([128, 3 * C], BF, tag=f"wt{ci}", name=f"wt{ci}")
                eng = nc.sync if ci == 0 else nc.scalar
                eng.dma_start(t[:, 0:512], w_d[128 * ci:128 * (ci + 1), 0:512])
                wt.append(t)
            w_sb = {}
            for i, nm in enumerate(("q", "k", "v")):
                for ci in range(2):
                    w_sb[nm, ci] = wt[ci][:, 256 * i:256 * (i + 1)]

            xA = []
            for ci in range(2):
                t = xpool.tile([128, S], BF, tag=f"xA{ci}", name=f"xA{ci}")
                eng = nc.sync if ci == 0 else nc.scalar
                eng.dma_start(t[:, 0:512], xA_d[128 * ci:128 * (ci + 1), 0:512])
                xA.append(t)
            bias_t = cpool.tile([128, 16], F32, tag="bias")
            nc.sync.dma_start(bias_t[:], bias_d[:])
            warm = cpool.tile([128, 1], F32, tag="warm")
            nc.scalar.activation(warm[:], bias_t[:, 0:1], Exp, bias=0.0,
                                 scale=0.0)
            for ci in range(2):
                eng = nc.sync if ci == 0 else nc.scalar
                eng.dma_start(wt[ci][:, 512:768],
                              w_d[128 * ci:128 * (ci + 1), 512:768])
            for ci in range(2):
                eng = nc.sync if ci == 0 else nc.scalar
                eng.dma_start(xA[ci][:, 512:S],
                              xA_d[128 * ci:128 * (ci + 1), 512:S])
            xC = []
            for ci in range(2):
                t = xpool.tile([128, 1536], BF, tag=f"xC{ci}", name=f"xC{ci}")
                eng = nc.sync if ci == 0 else nc.scalar
                eng.dma_start(t[:], xC_d[128 * ci:128 * (ci + 1), :])
                xC.append(t)
            xB = []
            for ci in range(2):
                t = xpool.tile([128, 1024], BF, tag=f"xB{ci}", name=f"xB{ci}")
                eng = nc.sync if ci == 0 else nc.scalar
                eng.dma_start(t[:], xB_d[128 * ci:128 * (ci + 1), :])
                xB.append(t)

            # ---- projection targets ----
            qt_bf = [kpool.tile([128, S], BF, tag=f"qtbf{c}", name=f"qtbf{c}")
                     for c in range(2)]
            qt_f8 = kpool.tile([128, 2, S], F8, tag="qtf8", name="qtf8")
            ktA_bf = [kpool.tile([128, S], BF, tag=f"kAbf{c}", name=f"kAbf{c}")
                      for c in range(2)]
            ktA_f8 = kpool.tile([128, 2, S], F8, tag="kAf8", name="kAf8")
            ktA_lo = kpool.tile([128, 2, S], F8, tag="kAlo", name="kAlo")
            ktB_f8 = kpool.tile([128, 2, 1024], F8, tag="kBf8", name="kBf8")
            ktC_f8 = kpool.tile([128, 2, 1536], F8, tag="kCf8", name="kCf8")
            ktB_bf = ktC_bf = None
            if no_f8:
                ktB_bf = [kpool.tile([128, 1024], BF, tag=f"kBbf{c}",
                                     name=f"kBbf{c}") for c in range(2)]
                ktC_bf = [kpool.tile([128, 1536], BF, tag=f"kCbf{c}",
                                     name=f"kCbf{c}") for c in range(2)]

            # persistent v slabs: [128, 16, 257] bf16 per job, col 256 = 1.0
            vsl = {}
            for jn in ("C", "B", "A"):
                v = vpool.tile([128, 16, 257], BF, tag=f"v{jn}", name=f"v{jn}")
                vsl[jn] = v
                nc.gpsimd.memset(v[:, 0:16, 256:257], 1.0)

            def vext(jn, t):
                return vsl[jn][:, t, :]

            # ---- projections (bf16) ----
            # psum -> bf16 slab (DVE) -> fp8 slab (Pool, SBUF->SBUF), or
            # psum -> fp8 slab directly on Act for fp8-only slabs.
            def proj_cols(dst_bf, dst_f8, wkey, src, c0, c1, dst_lo=None):
                for co in range(2):
                    ps = ps_proj.tile([128, 512], F32, tag="proj")
                    for ci in range(2):
                        nc.tensor.matmul(
                            ps[:, 0:c1 - c0],
                            w_sb[wkey, ci][:, 128 * co:128 * (co + 1)],
                            src[ci][:, c0:c1], start=(ci == 0), stop=(ci == 1))
                    if dst_bf is not None:
                        with tc.high_priority(offset=20):
                            nc.vector.tensor_copy(dst_bf[co][:, c0:c1],
                                                  ps[:, 0:c1 - c0])
                        if dst_f8 is not None:
                            if co == 0:
                                nc.gpsimd.tensor_copy(dst_f8[:, co, c0:c1],
                                                      dst_bf[co][:, c0:c1])
                            else:
                                nc.scalar.copy(dst_f8[:, co, c0:c1],
                                               dst_bf[co][:, c0:c1])
                            if dst_lo is not None:
                                eng = nc.vector if co == 0 else nc.gpsimd
                                eng.tensor_sub(dst_lo[:, co, c0:c1],
                                               dst_bf[co][:, c0:c1],
                                               dst_f8[:, co, c0:c1])
                    elif dst_f8 is not None:
                        nc.vector.tensor_copy(dst_f8[:, co, c0:c1],
                                              ps[:, 0:c1 - c0])

            def proj_v(jn, m, stat_aps2):
                """Project v tiles 2m, 2m+1 of job jn; one paired copy.
                C/B copies go on Act (idle during prep); A's on DVE."""
                ps = ps_proj.tile([128, 2, 256], F32, tag="proj", name="psv")
                for h in range(2):
                    for ci in range(2):
                        nc.tensor.matmul(ps[:, h, :], stat_aps2[h][ci],
                                         w_sb["v", ci][:],
                                         start=(ci == 0), stop=(ci == 1))
                nc.vector.tensor_copy(vsl[jn][:, 2 * m:2 * m + 2, 0:256],
                                      ps[:])

            # queries + A keys first (only need xA); then C, B keys; then V

            proj_cols(qt_bf, qt_f8, "q", xA, 0, 512)
            proj_cols(ktA_bf, ktA_f8, "k", xA, 0, 512, dst_lo=ktA_lo)
            for qc in range(1, 4):
                proj_cols(qt_bf, qt_f8, "q", xA, 512 * qc, 512 * (qc + 1))
                proj_cols(ktA_bf, ktA_f8, "k", xA, 512 * qc, 512 * (qc + 1),
                          dst_lo=ktA_lo)
            for kc in range(3):
                proj_cols(ktC_bf, ktC_f8, "k", xC, 512 * kc, 512 * (kc + 1))
            for kc in range(2):
                proj_cols(ktB_bf, ktB_f8, "k", xB, 512 * kc, 512 * (kc + 1))

            def xa_strided(t, base, step):
                off = (t - base) * 128 * step
                return [xA[ci][:, off:off + 128 * step:step] for ci in range(2)]

            def v_stat(jn, t):
                if jn == "C":
                    if t < 12:
                        return [xC[ci][:, 128 * t:128 * (t + 1)]
                                for ci in range(2)]
                    return xa_strided(t, 12, 4)
                if jn == "B":
                    if t < 8:
                        return [xB[ci][:, 128 * t:128 * (t + 1)]
                                for ci in range(2)]
                    return xa_strided(t, 8, 2)
                return [xA[ci][:, 128 * t:128 * (t + 1)] for ci in range(2)]


            for jn in ("C", "B"):
                for m in range(8):
                    proj_v(jn, m, [v_stat(jn, 2 * m), v_stat(jn, 2 * m + 1)])

            # ---- constants ----
            ones_t = cpool.tile([128, 128], F32, tag="ones")
            nc.gpsimd.memset(ones_t[:], 1.0)
            mtri_f = cpool.tile([128, 128], F32, tag="mtri_f")
            nc.gpsimd.affine_select(
                out=mtri_f[:], in_=ones_t[:],
                compare_op=mybir.AluOpType.is_ge,
                fill=0.0, base=0,
                pattern=[[1, 128]], channel_multiplier=-1,
            )
            mtri = cpool.tile([128, 128], BF, tag="mtri")
            nc.vector.tensor_copy(mtri[:], mtri_f[:])

            pmats = {}
            for stride, u in [(2, 0), (2, 1), (4, 0), (4, 1), (4, 2), (4, 3)]:
                pf = cpool.tile([128, 128], F32, tag=f"pmf{stride}_{u}",
                                name=f"pmf{stride}_{u}")
                nc.gpsimd.affine_select(
                    out=pf[:], in_=ones_t[:],
                    compare_op=mybir.AluOpType.is_equal,
                    fill=0.0, base=128 * u,
                    pattern=[[1, 128]], channel_multiplier=-stride,
                )
                pm = cpool.tile([128, 128], MMF, tag=f"pm{stride}_{u}",
                                name=f"pm{stride}_{u}")
                nc.vector.tensor_copy(pm[:], pf[:])
                pmats[stride, u] = pm

            # ---- attention ----
            def kt_f8_ap(jn, kt):
                if jn == "A":
                    return ktA_f8[:, :, 128 * kt:128 * (kt + 1)]
                if jn == "B":
                    if kt < 8:
                        return ktB_f8[:, :, 128 * kt:128 * (kt + 1)]
                    o = 256 * (kt - 8)
                    return ktA_f8[:, :, o:o + 256:2]
                if kt < 12:
                    return ktC_f8[:, :, 128 * kt:128 * (kt + 1)]
                o = 512 * (kt - 12)
                return ktA_f8[:, :, o:o + 512:4]

            def kt_bf_ap(jn, kt, ci):
                if jn == "A":
                    return ktA_bf[ci][:, 128 * kt:128 * (kt + 1)]
                if jn == "B":
                    if kt < 8:
                        return ktB_bf[ci][:, 128 * kt:128 * (kt + 1)]
                    o = 256 * (kt - 8)
                    return ktA_bf[ci][:, o:o + 256:2]
                if kt < 12:
                    return ktC_bf[ci][:, 128 * kt:128 * (kt + 1)]
                o = 512 * (kt - 12)
                return ktA_bf[ci][:, o:o + 512:4]

            def q_bf_ap(jn, i, qoff, ci):
                if jn == "A":
                    c0 = 512 * i + qoff
                    return qt_bf[ci][:, c0:512 * (i + 1)]
                if jn == "B":
                    c0 = 1024 * i + 2 * qoff
                    return qt_bf[ci][:, c0:1024 * (i + 1):2]
                return qt_bf[ci][:, 4 * qoff:S:4]

            def q_f8_ap(jn, i, qoff):
                if jn == "A":
                    c0 = 512 * i + qoff
                    return qt_f8[:, :, c0:512 * (i + 1)]
                if jn == "B":
                    c0 = 1024 * i + 2 * qoff
                    return qt_f8[:, :, c0:1024 * (i + 1):2]
                return qt_f8[:, :, 4 * qoff:S:4]

            def is_f8(jn, i, kt):
                if no_f8:
                    return False
                if jn == "A":
                    return i >= 1
                if jn == "B":
                    return i == 1 or kt < 8
                return kt < 12

            def bias_ap(jn, pair):
                if jn == "B" and pair < 4:
                    return bias_t[:, pair:pair + 1]
                if jn == "C" and pair < 6:
                    return bias_t[:, 4 + pair:5 + pair]
                return 0.0

            jobs = {"A": dict(P=0), "B": dict(P=8), "C": dict(P=12)}
            stB = [spool.tile([128, 258], MMF, tag=f"stB{t}", name=f"stB{t}")
                   for t in range(8)]
            stC = [spool.tile([128, 258], MMF, tag=f"stC{t}", name=f"stC{t}")
                   for t in range(4)]
            # stage col 257 is a dummy for the even-free-size f32r rule; it
            # only ever flows to psum col 257, which is never read.
            out_r = out_d.rearrange("(t p) c -> t p c", p=128)

            def scores_block(jn, i):
                """Emit scores+exp+mask for block i; return probs pair list."""
                P = jobs[jn]["P"]
                nkt = P + 4 * i + 4
                pbs = []
                for m in range(nkt // 2):
                    sc = ps_sc.tile([128, 2, 512], F32, tag="sc")
                    diag = False
                    qoffs = []
                    for h in range(2):
                        kt = 2 * m + h
                        jd = kt - (P + 4 * i)
                        qoff = max(0, 128 * jd)
                        qoffs.append(qoff)
                        diag = diag or jd >= 0
                        if is_f8(jn, i, kt):
                            klo = (ktA_lo[:, :, 128 * kt:128 * (kt + 1)]
                                   if jn == "A" else None)
                            nc.tensor.matmul(
                                sc[:, h, qoff:512], kt_f8_ap(jn, kt),
                                q_f8_ap(jn, i, qoff),
                                start=True, stop=(klo is None), perf_mode=DR)
                            if klo is not None:
                                nc.tensor.matmul(
                                    sc[:, h, qoff:512], klo,
                                    q_f8_ap(jn, i, qoff),
                                    start=False, stop=True, perf_mode=DR)
                        else:
                            for ci in range(2):
                                nc.tensor.matmul(
                                    sc[:, h, qoff:512], kt_bf_ap(jn, kt, ci),
                                    q_bf_ap(jn, i, qoff, ci),
                                    start=(ci == 0), stop=(ci == 1))
                    pb = ppool.tile([128, 2, 512], BF, tag="pb")
                    if sum(qoffs) > 256:
                        for h in range(2):
                            qoff = qoffs[h]
                            nc.scalar.activation(
                                pb[:, h, qoff:512], sc[:, h, qoff:512], Exp,
                                bias=0.0, scale=SCALE)
                    else:
                        nc.scalar.activation(pb[:], sc[:], Exp,
                                             bias=bias_ap(jn, m), scale=SCALE)
                    if diag:
                        for h in range(2):
                            jd = 2 * m + h - (P + 4 * i)
                            if jd >= 0:
                                qoff = qoffs[h]
                                nc.gpsimd.tensor_mul(
                                    pb[:, h, qoff:qoff + 128],
                                    pb[:, h, qoff:qoff + 128], mtri[:])
                    pbs.append(pb)
                return pbs

            def u_chain(jn, pbs, i, j, pre=()):
                P = jobs[jn]["P"]
                nk = P + 4 * i + j + 1
                u = ps_u.tile([128, 258], F32, tag="u")
                for n, (pm, st) in enumerate(pre):
                    nc.tensor.matmul(u[:], pm[:], st[:],
                                     start=(n == 0), stop=False)
                for kt in range(nk):
                    nc.tensor.matmul(
                        u[:, 0:257],
                        pbs[kt // 2][:, kt % 2, 128 * j:128 * (j + 1)],
                        vext(jn, kt),
                        start=(not pre and kt == 0), stop=(kt == nk - 1))
                return u

            # Emission order tuned so Act's exp queue (C, B0, B1) stays
            # ahead of the PE chains that consume the probs, while A's
            # scores fill PE gaps.
            for m in range(2):
                proj_v("A", m, [v_stat("A", 2 * m), v_stat("A", 2 * m + 1)])
            pbsC = scores_block("C", 0)
            pbA = {0: scores_block("A", 0)}
            pbsB = {0: scores_block("B", 0), 1: scores_block("B", 1)}
            for j in range(4):
                u = u_chain("C", pbsC, 0, j)
                with tc.high_priority(offset=20):
                    nc.vector.tensor_copy(stC[j][:, 0:257], u[:, 0:257])
            for j in range(4):
                u = u_chain("B", pbsB[0], 0, j)
                with tc.high_priority(offset=20):
                    nc.vector.tensor_copy(stB[j][:, 0:257], u[:, 0:257])
            for j in range(4):
                u = u_chain("B", pbsB[1], 1, j)
                with tc.high_priority(offset=20):
                    nc.vector.tensor_copy(stB[4 + j][:, 0:257], u[:, 0:257])
            pbA[1] = scores_block("A", 1)

            # A job ascending; next block's scores + V tiles emitted before
            # this block's chains.  pb pool slot reuse: A3's tiles wrap onto
            # A0's slots (freed by chains t0-3, emitted earlier) -- safe with
            # bufs=40.
            for i in range(4):
                for m in (2 * i + 2, 2 * i + 3):
                    if m < 8:
                        proj_v("A", m, [v_stat("A", 2 * m),
                                        v_stat("A", 2 * m + 1)])
                if i < 2:
                    pbA[i + 2] = scores_block("A", i + 2)
                for j in (range(4) if i < 3 else (3, 2, 1, 0)):
                    t = 4 * i + j
                    u = u_chain("A", pbA[i], i, j,
                                pre=((pmats[4, t % 4], stC[t // 4]),
                                     (pmats[2, t % 2], stB[t // 2])))
                    rec = rpool.tile([128, 1], F32, tag="rec")
                    nc.vector.reciprocal(rec[:], u[:, 256:257])
                    fin = fpool.tile([128, 256], F32, tag="fin")
                    nc.vector.tensor_scalar_mul(fin[:], u[:, 0:256], rec[:])
                    eng = nc.sync if t % 2 == 0 else nc.scalar
                    eng.dma_start(out_r[t], fin[:])

    nc.compile()
    return nc


def _get_program():
    global _PROG
    if _PROG is None:
        _PROG = _build_program()
    return _PROG


def make_in_maps(x, Wq, Wk, Wv):
    """Host-side sharding: gather / transpose / zero-pad + bf16 transport."""
    import concourse.mybir as mybir
    BFNP = mybir.dt.np(mybir.dt.bfloat16)

    x = np.asarray(x, dtype=np.float32)
    w = np.concatenate([np.asarray(Wq, dtype=np.float32),
                        np.asarray(Wk, dtype=np.float32),
                        np.asarray(Wv, dtype=np.float32)], axis=1)
    w = np.ascontiguousarray(w).astype(BFNP)
    in_maps = []
    for d in range(NCORES):
        b, q = divmod(d, 4)
        xA = np.ascontiguousarray(x[b, 2048 * q:2048 * (q + 1), :].T)

        seg = 4096 * (q // 2)
        if q % 2 == 1:
            xB = np.ascontiguousarray(x[b, seg:seg + 2048:2, :].T)
        else:
            xB = np.zeros((C, 1024), np.float32)

        r0c = 512 * q
        grp4 = x[b, 0:8192:4, :]
        rowsC = np.concatenate(
            [grp4[0:r0c], np.zeros((1536 - r0c, C), np.float32)], axis=0)
        xC = np.ascontiguousarray(rowsC.T)

        bias = np.zeros((128, 16), np.float32)
        if q % 2 == 0:
            bias[:, 0:4] = NEG          # B prefix pairs 0..3 masked
        for p in range(6):
            if p >= 2 * q:
                bias[:, 4 + p] = NEG    # C prefix pairs >= 2q masked

        in_maps.append({"xA": xA.astype(BFNP), "xB": xB.astype(BFNP),
                        "xC": xC.astype(BFNP), "w": w, "bias": bias})
    return in_maps


def kernel(x, Wq, Wk, Wv):
    from concourse.bass_utils import run_bass_kernel_spmd

    nc = _get_program()
    in_maps = make_in_maps(x, Wq, Wk, Wv)
    res = run_bass_kernel_spmd(nc, in_maps, core_ids=list(range(NCORES)))
    out = np.empty((B, N, C), np.float32)
    for d in range(NCORES):
        b, q = divmod(d, 4)
        out[b, 2048 * q:2048 * (q + 1), :] = res.results[d]["out"]
    return out
